# revision 1
# baseline (speedup 1.0000x reference)
"""CorrelationLayer1D Trainium2 Bass kernel.

Computes out[b, d, h, w] = sum_c x_1[b,c,h,w] * x2p[b,c,h,w+d] for d in [0, 41),
where x2p is x_2 width-padded by (8, 32).  Inputs [4,128,160,320] f32.

Sharding: data-parallel over H = 160 = 8*20 (correlation runs along W only, so
H-sharding needs no halo).  Per core, rows are processed in chunks of HC=10:

  per (b, h) row, per w-block (128/128/64):
    PE computes a block-diagonal Gram: two M=64 matmuls (lo: cols [w0,w0+64) x
    window [w0,w0+104); hi: cols [w0+64,w0+128) x window [w0+64,w0+168)) land
    as one compacted [128,104] PSUM tile - the 41-wide correlation band of
    partition i lives at free offset (i mod 64)+d.
  ScalarE copies each Gram into a per-chunk SBUF atlas [128, HC*312].
  Per chunk, 5 DMAs write the atlas to DRAM scratch and 5 skewed reloads
  (flat DRAM-side APs, step 105 = row-pitch+1) extract the band as
  S[i, (h,d)] - DRAM APs allow arbitrary affine steps; SBUF-side per-partition
  skews are not expressible, which is why the round-trip exists.
  PE transposes each row's S [wb,41] -> T[d, i]; VectorE copies T into a
  per-chunk assembly tile [41, HC*320]; one strided DMA per chunk stores it.
"""

import sys

import numpy as np

try:
    import concourse.bass as bass  # noqa: F401
except ImportError:
    sys.path.insert(0, "/opt/trn_rl_repo")

import concourse.bass as bass
import concourse.tile as tile
from concourse import bacc, masks, mybir
from concourse.ap import AP
from concourse.bass_utils import run_bass_kernel_spmd

MAX_DISP = 40
D = MAX_DISP + 1  # 41 displacements
PAD_L = 8
PAD_R = 32
B, C, H, W = 4, 128, 160, 320
N_CORES = 8
HS = H // N_CORES  # 20 h-rows per core
WP = W + PAD_L + PAD_R  # 360
WBLOCKS = [(0, 128), (128, 128), (256, 64)]
GW = 104  # compacted gram width per block: 64 + MAX_DISP
AW = 3 * GW  # atlas width per row: 312

F32 = mybir.dt.float32


def build_kernel(b_dim=B, hs=HS, hc=None):
    if hc is None:
        hc = 10 if hs % 10 == 0 else hs
    assert hs % hc == 0
    nchunks = hs // hc

    nc = bacc.Bacc("TRN2", target_bir_lowering=False, debug=False)
    x1e = nc.declare_dram_parameter("x1", [b_dim, C, hs, W], F32, isOutput=False)
    x2e = nc.declare_dram_parameter("x2", [b_dim, C, hs, W], F32, isOutput=False)
    oute = nc.declare_dram_parameter("out", [b_dim, D, hs, W], F32, isOutput=True)

    with tile.TileContext(nc) as tc:
        with (
            tc.tile_pool(name="const", bufs=1) as const_pool,
            tc.tile_pool(name="xin", bufs=3) as xin_pool,
            tc.tile_pool(name="atlas", bufs=2) as atlas_pool,
            tc.tile_pool(name="sbig", bufs=2) as sbig_pool,
            tc.tile_pool(name="asm", bufs=2) as asm_pool,
            tc.tile_pool(name="psum_g", bufs=4, space="PSUM") as psum_g,
            tc.tile_pool(name="psum_t", bufs=3, space="PSUM") as psum_t,
            tc.tile_pool(name="scratch", bufs=6, space="DRAM") as scratch_pool,
        ):
            identity = const_pool.tile([128, 128], F32)
            masks.make_identity(nc, identity[:])

            for b in range(b_dim):
                for ci in range(nchunks):
                    h0 = ci * hc
                    x1b = xin_pool.tile([C, hc * W], F32, tag="x1b")
                    nc.sync.dma_start(
                        x1b[:].rearrange("p (h w) -> p h w", w=W),
                        x1e[b, :, h0 : h0 + hc, :],
                    )
                    x2b = xin_pool.tile([C, hc * WP], F32, tag="x2b")
                    x2b3 = x2b[:].rearrange("p (h w) -> p h w", w=WP)
                    nc.gpsimd.memset(x2b3[:, :, 0:PAD_L], 0.0)
                    nc.gpsimd.memset(x2b3[:, :, PAD_L + W : WP], 0.0)
                    nc.sync.dma_start(
                        x2b3[:, :, PAD_L : PAD_L + W], x2e[b, :, h0 : h0 + hc, :]
                    )

                    atlas = atlas_pool.tile([C, hc * AW], F32, tag="atlas")
                    abatch = asm_pool.tile([D, hc * W], F32, tag="abatch")
                    sbig = [
                        sbig_pool.tile([128, hc * D], F32, tag="sbig0", name=f"sbig0_{b}_{ci}"),
                        sbig_pool.tile([128, hc * D], F32, tag="sbig1", name=f"sbig1_{b}_{ci}"),
                        sbig_pool.tile([64, hc * D], F32, tag="sbig2", name=f"sbig2_{b}_{ci}"),
                    ]

                    # Gram compute + PSUM->SBUF atlas copies, row by row.
                    for hh in range(hc):
                        o1 = hh * W
                        o2 = hh * WP
                        for kblk, (w0, wb) in enumerate(WBLOCKS):
                            gram_ps = psum_g.tile([wb, GW], F32, tag="gram")
                            nc.tensor.matmul(
                                gram_ps[0:64, :],
                                x1b[:, o1 + w0 : o1 + w0 + 64],
                                x2b[:, o2 + w0 : o2 + w0 + GW],
                                start=True,
                                stop=True,
                                tile_position=(0, 0),
                            )
                            if wb == 128:
                                nc.tensor.matmul(
                                    gram_ps[64:128, :],
                                    x1b[:, o1 + w0 + 64 : o1 + w0 + 128],
                                    x2b[:, o2 + w0 + 64 : o2 + w0 + 64 + GW],
                                    start=True,
                                    stop=True,
                                    tile_position=(0, 64),
                                )
                            aoff = kblk * hc * GW + hh * GW
                            nc.scalar.copy(
                                atlas[0:wb, aoff : aoff + GW],
                                gram_ps[:],
                            )

                    # Scratch round-trip: rectangular store, skewed reload.
                    for kblk, half in [(0, 0), (0, 1), (1, 0), (1, 1), (2, 0)]:
                        scr = scratch_pool.tile([64, hc * GW], F32, tag="scr")
                        nc.sync.dma_start(
                            scr[:],
                            atlas[
                                half * 64 : (half + 1) * 64,
                                kblk * hc * GW : (kblk + 1) * hc * GW,
                            ],
                        )

                        scr_ap = scr[:]
                        diag = AP(
                            tensor=scr_ap.tensor,
                            offset=scr_ap.offset,
                            ap=[[hc * GW + 1, 64], [GW, hc], [1, D]],
                        )
                        sb = sbig[kblk]
                        dstp = sb[half * 64 : (half + 1) * 64, :].rearrange(
                            "p (h d) -> p h d", d=D
                        )
                        nc.scalar.dma_start(dstp, diag)

                    # Transpose each row-block band to [d, w] and assemble.
                    for hh in range(hc):
                        for kblk, (w0, wb) in enumerate(WBLOCKS):
                            sb = sbig[kblk]
                            t_ps = psum_t.tile([D, wb], F32, tag="t_ps")
                            nc.tensor.matmul(
                                t_ps[:],
                                sb[0:wb, hh * D : (hh + 1) * D],
                                identity[0:wb, 0:wb],
                                start=True,
                                stop=True,
                                is_transpose=True,
                            )
                            nc.vector.tensor_copy(
                                abatch[:, hh * W + w0 : hh * W + w0 + wb], t_ps[:]
                            )

                    nc.scalar.dma_start(
                        oute[b, :, h0 : h0 + hc, :],
                        abatch[:].rearrange("d (h w) -> d h w", w=W),
                    )

    nc.finalize()
    return nc


_compiled = {}


def _get_kernel(b_dim, hs):
    key = (b_dim, hs)
    if key not in _compiled:
        _compiled[key] = build_kernel(b_dim, hs)
    return _compiled[key]


def kernel(x_1: np.ndarray, x_2: np.ndarray) -> np.ndarray:
    assert x_1.shape == (B, C, H, W) and x_2.shape == (B, C, H, W)
    x_1 = np.ascontiguousarray(x_1, dtype=np.float32)
    x_2 = np.ascontiguousarray(x_2, dtype=np.float32)
    nc = _get_kernel(B, HS)
    in_maps = [
        {
            "x1": np.ascontiguousarray(x_1[:, :, i * HS : (i + 1) * HS, :]),
            "x2": np.ascontiguousarray(x_2[:, :, i * HS : (i + 1) * HS, :]),
        }
        for i in range(N_CORES)
    ]
    res = run_bass_kernel_spmd(nc, in_maps, core_ids=list(range(N_CORES))).results
    out = np.concatenate([res[i]["out"] for i in range(N_CORES)], axis=2)
    return out



# revision 6
# speedup vs baseline: 1.8951x; 1.8951x over previous
"""CorrelationLayer1D Trainium2 Bass kernel (scratch-free v2).

Computes out[b, d, h, w] = sum_c x_1[b,c,h,w] * x2p[b,c,h,w+d] for d in [0, 41),
where x2p is x_2 width-padded by (8, 32).  Inputs [4,128,160,320] f32.

Sharding: data-parallel over H = 160 = 8*20 (correlation runs along W only, so
H-sharding needs no halo).  Per core, rows are processed in chunks of HC=10.

Per (b, h) row the W=320 axis splits into blocks M = 128/128/64 with x2 windows
168/168/104.  bf16 Gram matmuls land in PSUM; Act/DVE copies compact them into
per-chunk bf16 atlases [M, HC*win].  The diagonal band S[i, (h,d)] = G[i, h, i+d]
is extracted ON-CHIP by gpsimd.local_scatter with per-partition indices
(j -> j - i, out-of-band j mapped to -1 = dropped) - no DRAM scratch round-trip.
PE transposes 3-row groups S[128, 123] -> T[(h,d), w-block] (bf16 PSUM), one
Act/DVE copy casts T to an fp32 SBUF group tile [123, 320], and a single strided
DMA per group writes out[b, :, h:h+3, :].
"""

import sys

import numpy as np

try:
    import concourse.bass as bass  # noqa: F401
except ImportError:
    sys.path.insert(0, "/opt/trn_rl_repo")

import concourse.bass as bass
import concourse.tile as tile
from concourse import bacc, masks, mybir
from concourse.ap import AP
from concourse.bass_utils import run_bass_kernel_spmd

MAX_DISP = 40
D = MAX_DISP + 1  # 41 displacements
PAD_L = 8
PAD_R = 32
B, C, H, W = 4, 128, 160, 320
N_CORES = 8
HS = H // N_CORES  # 20 h-rows per core
WP = W + PAD_L + PAD_R  # 360
# (w0, M, window) per block; window = M + MAX_DISP
WBLOCKS = [(0, 128, 168), (128, 128, 168), (256, 64, 104)]

F32 = mybir.dt.float32
BF16 = mybir.dt.bfloat16
I16 = mybir.dt.int16


def build_kernel(b_dim=B, hs=HS, hc=10):
    assert hs % hc == 0
    nchunks = hs // hc

    nc = bacc.Bacc("TRN2", target_bir_lowering=False, debug=False)
    x1e = nc.declare_dram_parameter("x1", [b_dim, C, hs, W], F32, isOutput=False)
    x2e = nc.declare_dram_parameter("x2", [b_dim, C, hs, W], F32, isOutput=False)
    oute = nc.declare_dram_parameter("out", [b_dim, D, hs, W], F32, isOutput=True)

    with tile.TileContext(nc) as tc:
        with (
            tc.tile_pool(name="const", bufs=1) as const_pool,
            tc.tile_pool(name="xf", bufs=2) as xf_pool,
            tc.tile_pool(name="xbf", bufs=2) as xbf_pool,
            tc.tile_pool(name="atl", bufs=2) as atl_pool,
            tc.tile_pool(name="sband", bufs=2) as s_pool,
            tc.tile_pool(name="abg", bufs=3) as abg_pool,
            tc.tile_pool(name="psum_g", bufs=4, space="PSUM") as psum_g,
            tc.tile_pool(name="psum_g2", bufs=2, space="PSUM") as psum_g2,
            tc.tile_pool(name="psum_t", bufs=2, space="PSUM") as psum_t,
        ):
            identity = const_pool.tile([128, 128], BF16)
            masks.make_identity(nc, identity[:])

            # Per-partition scatter indices: for gram partition i, element
            # (h, j) goes to h*D + (j - i) when 0 <= j - i <= MAX_DISP, else -1.
            idx_tiles = {}
            for mth, win in ((128, 168), (64, 104)):
                idx = const_pool.tile([mth, hc * win], I16, name=f"idx_{mth}")
                nc.gpsimd.iota(
                    idx[:],
                    pattern=[[D, hc], [1, win]],
                    base=0,
                    channel_multiplier=-1,
                )
                # keep where j - i >= 0
                nc.gpsimd.affine_select(
                    out=idx[:],
                    in_=idx[:],
                    pattern=[[0, hc], [1, win]],
                    compare_op=mybir.AluOpType.is_ge,
                    fill=-1,
                    base=0,
                    channel_multiplier=-1,
                )
                # keep where MAX_DISP - (j - i) >= 0
                nc.gpsimd.affine_select(
                    out=idx[:],
                    in_=idx[:],
                    pattern=[[0, hc], [-1, win]],
                    compare_op=mybir.AluOpType.is_ge,
                    fill=-1,
                    base=MAX_DISP,
                    channel_multiplier=1,
                )
                idx_tiles[mth] = idx

            for b in range(b_dim):
                for ci in range(nchunks):
                    h0 = ci * hc

                    # ---- load fp32 inputs (contiguous 12.8KB runs) ----
                    x1f = xf_pool.tile([C, hc * W], F32, tag="x1f")
                    nc.sync.dma_start(
                        x1f[:].rearrange("p (h w) -> p h w", w=W),
                        x1e[b, :, h0 : h0 + hc, :],
                    )
                    x2f = xf_pool.tile([C, hc * W], F32, tag="x2f")
                    nc.sync.dma_start(
                        x2f[:].rearrange("p (h w) -> p h w", w=W),
                        x2e[b, :, h0 : h0 + hc, :],
                    )

                    # ---- convert to bf16 (x2 into padded layout) ----
                    x1b = xbf_pool.tile([C, hc * W], BF16, tag="x1b")
                    nc.scalar.copy(x1b[:], x1f[:])
                    x2b = xbf_pool.tile([C, hc * WP], BF16, tag="x2b")
                    x2b3 = x2b[:].rearrange("p (h w) -> p h w", w=WP)
                    x2f3 = x2f[:].rearrange("p (h w) -> p h w", w=W)
                    nc.gpsimd.memset(x2b3[:, :, 0:PAD_L], 0.0)
                    nc.gpsimd.memset(x2b3[:, :, PAD_L + W : WP], 0.0)
                    half = hc // 2
                    nc.scalar.copy(
                        x2b3[:, 0:half, PAD_L : PAD_L + W], x2f3[:, 0:half, :]
                    )
                    nc.vector.tensor_copy(
                        x2b3[:, half:hc, PAD_L : PAD_L + W], x2f3[:, half:hc, :]
                    )

                    # ---- Gram matmuls -> PSUM -> bf16 atlases ----
                    atls = [
                        atl_pool.tile([128, hc * 168], BF16, tag="a0", name=f"a0_{b}_{ci}"),
                        atl_pool.tile([128, hc * 168], BF16, tag="a1", name=f"a1_{b}_{ci}"),
                        atl_pool.tile([64, hc * 104], BF16, tag="a2", name=f"a2_{b}_{ci}"),
                    ]
                    for h in range(0, hc, 2):
                        for k in (0, 1):
                            w0, _, win = WBLOCKS[k]
                            ps = psum_g.tile([128, 2 * 168], F32, tag="g01")
                            for r in (0, 1):
                                nc.tensor.matmul(
                                    ps[:, r * 168 : (r + 1) * 168],
                                    x1b[:, (h + r) * W + w0 : (h + r) * W + w0 + 128],
                                    x2b[:, (h + r) * WP + w0 : (h + r) * WP + w0 + win],
                                    start=True,
                                    stop=True,
                                )
                            if k == 0:
                                nc.scalar.copy(
                                    atls[k][:, h * 168 : (h + 2) * 168], ps[:]
                                )
                            else:
                                nc.vector.tensor_copy(
                                    atls[k][:, h * 168 : (h + 2) * 168], ps[:]
                                )
                    w0, _, win = WBLOCKS[2]
                    for h in range(0, hc, 4):
                        rr = min(4, hc - h)
                        ps = psum_g2.tile([64, 4 * 104], F32, tag="g2")
                        for r in range(rr):
                            nc.tensor.matmul(
                                ps[:, r * 104 : (r + 1) * 104],
                                x1b[:, (h + r) * W + w0 : (h + r) * W + w0 + 64],
                                x2b[:, (h + r) * WP + w0 : (h + r) * WP + w0 + win],
                                start=True,
                                stop=True,
                            )
                        nc.vector.tensor_copy(
                            atls[2][:, h * 104 : (h + rr) * 104], ps[:, 0 : rr * 104]
                        )

                    # ---- on-chip diagonal band extraction (gpsimd) ----
                    sb = [
                        s_pool.tile([128, hc * D], BF16, tag="s0", name=f"s0_{b}_{ci}"),
                        s_pool.tile([128, hc * D], BF16, tag="s1", name=f"s1_{b}_{ci}"),
                        s_pool.tile([64, hc * D], BF16, tag="s2", name=f"s2_{b}_{ci}"),
                    ]
                    for k, (w0k, mk, wink) in enumerate(WBLOCKS):
                        nc.gpsimd.local_scatter(
                            sb[k][:],
                            atls[k][:],
                            idx_tiles[mk][:],
                            channels=mk,
                            num_elems=hc * D,
                            num_idxs=hc * wink,
                        )

                    # ---- PE transpose 3-row groups + fp32 out ----
                    h = 0
                    gi = 0
                    while h < hc:
                        g = min(3, hc - h)
                        gp = g * D
                        pst = psum_t.tile([3 * D, W], BF16, tag="t")
                        for k, (w0k, mk, wink) in enumerate(WBLOCKS):
                            nc.tensor.matmul(
                                pst[0:gp, w0k : w0k + mk],
                                sb[k][0:mk, h * D : (h + g) * D],
                                identity[0:mk, 0:mk],
                                start=True,
                                stop=True,
                                is_transpose=True,
                            )
                        abg = abg_pool.tile(
                            [3 * D, W], F32, tag="abg", name=f"abg_{b}_{ci}_{gi}"
                        )
                        if gi % 2 == 0:
                            nc.scalar.copy(abg[0:gp, :], pst[0:gp, :])
                        else:
                            nc.vector.tensor_copy(abg[0:gp, :], pst[0:gp, :])

                        ob = oute[b]
                        dst = AP(
                            tensor=ob.tensor,
                            offset=ob.offset + (h0 + h) * W,
                            ap=[[W, g], [hs * W, D], [1, W]],
                        )
                        nc.sync.dma_start(dst, abg[0:gp, :])
                        h += g
                        gi += 1

    nc.finalize()
    return nc


_compiled = {}


def _get_kernel(b_dim, hs):
    key = (b_dim, hs)
    if key not in _compiled:
        _compiled[key] = build_kernel(b_dim, hs)
    return _compiled[key]


def kernel(x_1: np.ndarray, x_2: np.ndarray) -> np.ndarray:
    assert x_1.shape == (B, C, H, W) and x_2.shape == (B, C, H, W)
    x_1 = np.ascontiguousarray(x_1, dtype=np.float32)
    x_2 = np.ascontiguousarray(x_2, dtype=np.float32)
    nc = _get_kernel(B, HS)
    in_maps = [
        {
            "x1": np.ascontiguousarray(x_1[:, :, i * HS : (i + 1) * HS, :]),
            "x2": np.ascontiguousarray(x_2[:, :, i * HS : (i + 1) * HS, :]),
        }
        for i in range(N_CORES)
    ]
    res = run_bass_kernel_spmd(nc, in_maps, core_ids=list(range(N_CORES))).results
    out = np.concatenate([res[i]["out"] for i in range(N_CORES)], axis=2)
    return out
